# revision 10
# baseline (speedup 1.0000x reference)
"""Trainium2 Bass kernel for nn_MultiHeadAttention (B=2, S=2048, D=1024, H=16).

Sharding: 8 cores = 2 (batch) x 4 (head groups of 4 heads / 256 dims).
Each core computes QKV projections for its head slice, attention for its 4
heads, and the partial output projection for its 256-dim slice of Wo's input.
Host sums the 4 partials per batch element (Megatron-style row-parallel Wo).

Steady-state design: the scalar engine (exp over 4 heads x 2048 x 2048
scores = 16.8M elems/core at 1 elem/lane/cycle @1.2GHz) is the pacer at
~1147ns per 128-t-chunk.  The PE stream per chunk is scores(c+1) lookahead
(2 concurrent K=64 row-group MMs), attnV(prev iter, c) (2 MMs M=65 with a
ones column computing the softmax denominator), plus ~1 "extra" unit
(projection / Wo work), ~950ns total, so exp never starves.

DRAM layouts are pre-swizzled on the host so every DMA line is 4-16KB
contiguous per partition (full HBM rate).  PSUM: scores 2x2 banks, attnV
accum 2x1, extras 2x1 = 8 banks.  Pt (exp output) is head-major [h][c][s]
so mask-muls are contiguous (2x DVE packed-bf16 rate) and attnV rhs slices
stay contiguous.  Bias-adds and V-copies run on GpSimd to keep the DVE at
its mask-mul + norm + output-staging budget.
"""

import sys

import numpy as np

try:
    import concourse.bass as bass
except ImportError:  # pragma: no cover
    sys.path.insert(0, "/opt/trn_rl_repo")
    import concourse.bass as bass

from concourse import bacc

import ml_dtypes

import concourse.tile as tile_mod
from concourse import mybir
from concourse.bass_utils import run_bass_kernel_spmd

BF16 = ml_dtypes.bfloat16
F32 = np.float32

B, S, D, H = 2, 2048, 1024, 16
DK = D // H            # 64
N_CORES = 8
HPC = 4                # heads per core
JC = HPC * DK          # 256 j-dims per core
SCALE = 1.0 / float(np.sqrt(DK))
NSB = S // 512         # 4 s-blocks
NC_T = S // 128        # 16 t-chunks
NIT = NSB * 2          # 8 (sb, pair) iterations

bf = mybir.dt.bfloat16
f32 = mybir.dt.float32


def _patch_drain():
    """This walrus build only accepts 1 sync-wait per instruction; the Tile
    exit drain carries one wait per pending proc. Split them across drains."""
    if getattr(tile_mod.TileContext, "_drain_patched", False):
        return
    import bass_rust

    def _drain_and_barrier(self, tick_clock, wait_clock):
        from concourse.tile import ScopedClock

        nc = self.nc
        drain_inst = nc.sync.drain()
        wait_clock.add_sem_waits(
            drain_inst.ins, ScopedClock({None: tick_clock.global_clock})
        )
        si = drain_inst.ins.sync_info
        waits = list(si.on_wait)
        if len(waits) > 1:
            drain_inst.ins.sync_info = bass_rust.SyncInfo(
                on_wait=[waits[0]], on_update=list(si.on_update)
            )
            for w in waits[1:]:
                d2 = nc.sync.drain()
                d2.ins.sync_info = bass_rust.SyncInfo(on_wait=[w], on_update=[])
        nc.all_engine_barrier()
        assert self.sems is not None
        popped = nc._tile_sem_poison_stack.pop()
        assert popped is self._sem_poison
        nc.clear_and_free_semaphores(list(self.sems.allocated().values()))
        nc.all_engine_barrier()

    tile_mod.TileContext._drain_and_barrier = _drain_and_barrier
    tile_mod.TileContext._drain_patched = True


def _emit(tc, T):
    nc = tc.nc
    Exp = mybir.ActivationFunctionType.Exp

    from contextlib import ExitStack

    with ExitStack() as ctx:
        persist = ctx.enter_context(tc.tile_pool(name="persist", bufs=1))

        # ---- preload the exp table set while DMAs stream ----
        junk_in = persist.tile([1, 8], f32, tag="junk_in")
        junk_out = persist.tile([1, 8], f32, tag="junk_out")
        nc.gpsimd.memset(junk_in[:], 0.0)
        nc.scalar.activation(junk_out[:], junk_in[:], Exp, scale=1.0)

        # ---- persistent tiles ----
        wq = persist.tile([128, 8 * JC], bf, tag="wq")
        wk = persist.tile([128, 8 * JC], bf, tag="wk")
        wv = persist.tile([128, 8 * JC], bf, tag="wv")
        wo = [persist.tile([128, D], bf, tag=f"wo{i}", name=f"wo{i}") for i in range(2)]
        biasqk = persist.tile([128, 4], f32, tag="biasqk")
        qpS = [
            [persist.tile([128, 512], bf, tag=f"qp{j}_{s}", name=f"qp{j}_{s}")
             for s in range(NSB)]
            for j in range(2)
        ]
        # kpT[pair][quarter]: [128 j, 512 t] (t-quarter q = t-chunks 4q..4q+3)
        kpT = [
            [persist.tile([128, 512], bf, tag=f"kpT{j}_{q}", name=f"kpT{j}_{q}")
             for q in range(NSB)]
            for j in range(2)
        ]
        VROW = HPC * 65
        vpc = [persist.tile([128, VROW], bf, tag=f"vp{c}", name=f"vp{c}")
               for c in range(NC_T)]
        concatT = [persist.tile([128, S], bf, tag=f"concatT{i}", name=f"concatT{i}")
                   for i in range(2)]

        wq_v = wq[:].rearrange("p (c j) -> p c j", c=8)
        wk_v = wk[:].rearrange("p (c j) -> p c j", c=8)
        wv_v = wv[:].rearrange("p (c j) -> p c j", c=8)

        # ---- streaming pools ----
        q_stream = ctx.enter_context(tc.tile_pool(name="q_stream", bufs=1))
        kv_stream = ctx.enter_context(tc.tile_pool(name="kv_stream", bufs=2))
        vstream = ctx.enter_context(tc.tile_pool(name="vstream", bufs=2))
        maskp = ctx.enter_context(tc.tile_pool(name="maskp", bufs=2))
        ptp = ctx.enter_context(tc.tile_pool(name="ptp", bufs=2))
        outp = ctx.enter_context(tc.tile_pool(name="outp", bufs=2))
        smallp = ctx.enter_context(tc.tile_pool(name="smallp", bufs=2))
        # PSUM: scores 2x2 banks + attnV accum 2x1 + extras 2x1 = 8 banks
        scp = ctx.enter_context(tc.tile_pool(name="scp", bufs=2, space="PSUM"))
        avp = ctx.enter_context(tc.tile_pool(name="avp", bufs=2, space="PSUM"))
        xp = ctx.enter_context(tc.tile_pool(name="xp", bufs=2, space="PSUM"))

        qtts = {}
        ktts = {}
        vtts = {}
        mtiles = {}

        def dma_w3(t, name):
            nc.sync.dma_start(
                t[:].rearrange("p (c j) -> p c j", c=8), T[name][:, :, :]
            )

        def emit_qdma(sb):
            t = q_stream.tile([128, 8 * 512], bf, tag="qTt", name=f"qTt{sb}")
            nc.sync.dma_start(
                t[:].rearrange("p (c s) -> p c s", c=8), T["qT4"][sb, :, :, :]
            )
            qtts[sb] = t[:].rearrange("p (c s) -> p c s", c=8)

        def emit_kdma(sb, eng):
            t = kv_stream.tile([128, 8 * 512], bf, tag="kTt", name=f"kTt{sb}")
            eng.dma_start(
                t[:].rearrange("p (c s) -> p c s", c=8), T["kT4"][sb, :, :, :]
            )
            ktts[sb] = t[:].rearrange("p (c s) -> p c s", c=8)

        def emit_vdma(tb):
            t = vstream.tile([128, 8 * 512], bf, tag="vTt", name=f"vTt{tb}")
            nc.gpsimd.dma_start(
                t[:].rearrange("p (c s) -> p c s", c=8), T["vT4"][tb, :, :, :]
            )
            vtts[tb] = t[:].rearrange("p (c t) -> p c t", c=8)

        def emit_mask_dma(sb):
            t = maskp.tile([128, NC_T * 512], bf, tag="mT", name=f"mT{sb}")
            nc.gpsimd.dma_start(
                t[:].rearrange("p (c s) -> p c s", c=NC_T),
                T["mask4"][sb, :, :, :],
            )
            mtiles[sb] = t

        # ---- prologue DMAs ----
        # sync queue: biasqk, wq, q0, wk, k0, wv, k2, k3, wo; q1-3 in-loop
        # gpsimd queue: k1, mask0, v0, v1; mask1-3/v2-3 in-loop
        nc.sync.dma_start(biasqk[:], T["biasqk"][:, :])
        dma_w3(wq, "wq3")
        emit_qdma(0)
        dma_w3(wk, "wk3")
        emit_kdma(0, nc.sync)
        emit_kdma(1, nc.gpsimd)
        dma_w3(wv, "wv3")
        emit_kdma(2, nc.sync)
        emit_kdma(3, nc.sync)
        for i in range(2):
            nc.sync.dma_start(wo[i][:], T["woT"][i * 128 : (i + 1) * 128, :])
        emit_mask_dma(0)
        emit_vdma(0)
        emit_vdma(1)

        # ---- projection emitters (PE part split into half-chains of 4 MMs) ----
        proj_ps = {}

        def emit_qproj(sb, jt, half):
            jsl = slice(jt * 128, (jt + 1) * 128)
            key = f"pq{sb}_{jt}"
            if half == 0:
                proj_ps[key] = xp.tile([128, 512], f32, tag="x", name=key)
            ps = proj_ps[key]
            for c in range(half * 4, half * 4 + 4):
                nc.tensor.matmul(
                    ps[:], wq_v[:, c, jsl], qtts[sb][:, c, :],
                    start=(c == 0), stop=(c == 7),
                )
            if half == 1:
                del proj_ps[key]
                nc.vector.tensor_scalar_add(
                    qpS[jt][sb][:], ps[:], biasqk[:, jt : jt + 1]
                )

        def emit_kproj(sb, jt, half):
            jsl = slice(jt * 128, (jt + 1) * 128)
            key = f"pk{sb}_{jt}"
            if half == 0:
                proj_ps[key] = xp.tile([128, 512], f32, tag="x", name=key)
            ps = proj_ps[key]
            for c in range(half * 4, half * 4 + 4):
                nc.tensor.matmul(
                    ps[:], wk_v[:, c, jsl], ktts[sb][:, c, :],
                    start=(c == 0), stop=(c == 7),
                )
            if half == 1:
                del proj_ps[key]
                nc.vector.tensor_scalar_add(
                    kpT[jt][sb][:], ps[:], biasqk[:, 2 + jt : 3 + jt]
                )

        def emit_vproj(chunk, half):
            # two consecutive chunks share one xp tile (256-col halves)
            tb, tt = chunk // 4, chunk % 4
            key = f"pv{chunk // 2}"
            if chunk % 2 == 0 and half == 0:
                proj_ps[key] = xp.tile([128, 512], f32, tag="x", name=key)
            ps = proj_ps[key]
            col = (chunk % 2) * 256
            for c in range(half * 4, half * 4 + 4):
                nc.tensor.matmul(
                    ps[:, col : col + 256],
                    vtts[tb][:, c, tt * 128 : (tt + 1) * 128],
                    wv_v[:, c, :],
                    start=(c == 0), stop=(c == 7),
                )
            if half == 1:
                vt = vpc[chunk]
                nc.gpsimd.memset(
                    vt[:].rearrange("p (h d) -> p h d", d=65)[:, :, 64:65], 1.0
                )
                nc.vector.tensor_copy(
                    vt[:].rearrange("p (h d) -> p h d", h=HPC)[:, :, 0:DK],
                    ps[:, col : col + 256].rearrange("p (h d) -> p h d", h=HPC),
                )
                if chunk % 2 == 1:
                    del proj_ps[key]

        def emit_wo_group(sb, st, mt):
            s0 = sb * 512 + st * 128
            msl = slice(mt * 512, (mt + 1) * 512)
            pw = xp.tile([128, 512], f32, tag="x", name=f"pw{sb}_{st}_{mt}")
            for kc in range(2):
                nc.tensor.matmul(
                    pw[:],
                    concatT[kc][:, s0 : s0 + 128],
                    wo[kc][:, msl],
                    start=(kc == 0), stop=(kc == 1),
                )
            ot = outp.tile([128, 512], f32, tag="ot", name=f"ot{sb}_{st}_{mt}")
            nc.vector.tensor_copy(ot[:], pw[:])
            nc.sync.dma_start(T["out_p"][s0 : s0 + 128, msl], ot[:])

        # ---- prologue PE work: k0/k1 then q0 projections ----
        for sb in range(2):
            for jt in range(2):
                emit_kproj(sb, jt, 0)
                emit_kproj(sb, jt, 1)
        for jt in range(2):
            emit_qproj(0, jt, 0)
            emit_qproj(0, jt, 1)

        # ---- attention pipeline ----
        it_list = [(sb, pair) for sb in range(NSB) for pair in range(2)]

        sc_tiles = {}

        def emit_scores(i, c):
            sb, pair = it_list[i]
            ps = scp.tile([128, 1024], f32, tag="sc", name=f"sc{i}_{c}")
            for h2 in range(2):
                psl = slice(h2 * 64, h2 * 64 + 64)
                nc.tensor.matmul(
                    ps[:, h2 * 512 : (h2 + 1) * 512],
                    kpT[pair][c // 4][psl, (c % 4) * 128 : (c % 4 + 1) * 128],
                    qpS[pair][sb][psl, :],
                    start=True, stop=True,
                )
            sc_tiles[(i, c)] = ps

        pt_views = {}

        def emit_exp(i, c):
            ps = sc_tiles.pop((i, c))
            ptv = pt_views[i]
            nc.scalar.activation(
                ptv[:, :, c, :],
                ps[:].rearrange("p (h s) -> p h s", h=2),
                Exp, scale=SCALE,
            )

        def emit_attnv(i, c, po2, start, stop):
            sb, pair = it_list[i]
            ptv = pt_views[i]
            for h2 in range(2):
                h = pair * 2 + h2
                nc.tensor.matmul(
                    po2[h2][0:65, :],
                    vpc[c][:, h * 65 : h * 65 + 65],
                    ptv[:, h2, c, :],
                    start=start, stop=stop,
                )

        def emit_mask(i, half):
            sb, _ = it_list[i]
            ptv = pt_views[i]
            mv = mtiles[sb][:].rearrange("p (c s) -> p c s", c=NC_T)
            csl = slice(half * 8, half * 8 + 8)
            for h2 in range(2):
                nc.vector.tensor_mul(
                    ptv[:, h2, csl, :], ptv[:, h2, csl, :], mv[:, csl, :]
                )

        def emit_norm(i, po2):
            sb, pair = it_list[i]
            sl = slice(sb * 512, (sb + 1) * 512)
            for h2 in range(2):
                po = po2[h2]
                rc0 = smallp.tile([1, 512], f32, tag="rc0", name=f"rc0_{i}_{h2}")
                nc.vector.tensor_copy(rc0[:], po[64:65, :])
                rc = smallp.tile([1, 512], f32, tag="rc", name=f"rc{i}_{h2}")
                nc.vector.reciprocal_approx_fast(rc[:], rc0[:])
                rb = smallp.tile([64, 512], f32, tag="rb", name=f"rb{i}_{h2}")
                nc.gpsimd.partition_broadcast(rb[:], rc[:], channels=64)
                nc.vector.tensor_mul(
                    concatT[pair][h2 * 64 : h2 * 64 + 64, sl], po[0:64, :], rb[:]
                )

        # extras: (kind, closure) units; kproj for k2/k3 (DMA-arrival order),
        # then all vproj units.  ~1 unit (4 MMs) fits a 1147ns exp slot next
        # to scores+attnV.
        extras = []
        for sb, jt in ((2, 0), (2, 1), (3, 0), (3, 1)):
            for half in range(2):
                extras.append(("k", lambda s=sb, j=jt, h=half: emit_kproj(s, j, h)))
        for ch in range(NC_T):
            for half in range(2):
                extras.append(("v", lambda c=ch, h=half: emit_vproj(c, h)))

        emit_scores(0, 0)
        po2 = None
        po2L = None
        for i in range(NIT):
            sb, pair = it_list[i]
            pt = ptp.tile([128, 2 * NC_T * 512], bf, tag="Pt", name=f"Pt{i}")
            pt_views[i] = pt[:].rearrange("p (h c s) -> p h c s", h=2, c=NC_T)
            if i > 0:
                po2 = [
                    avp.tile([128, 512], f32, tag="av", name=f"av{i - 1}_{h2}")
                    for h2 in range(2)
                ]
            if pair == 1 and i + 2 < NIT:
                # qpS[.][sb+1] needed when scores(i+2, 0) executes; insert after
                # any vproj units (tighter deadlines) but before Wo units.
                units = []
                for jt in range(2):
                    for half in range(2):
                        units.append(
                            ("q", lambda s=sb + 1, j=jt, h=half: emit_qproj(s, j, h))
                        )
                idx = 0
                for pos, (kind, _) in enumerate(extras):
                    if kind == "v":
                        idx = pos + 1
                extras[idx:idx] = units
            for c in range(NC_T):
                # scores lookahead (+1)
                if c < NC_T - 1:
                    emit_scores(i, c + 1)
                elif i + 1 < NIT:
                    emit_scores(i + 1, 0)
                emit_exp(i, c)
                if i > 0:
                    emit_attnv(i - 1, c, po2, c == 0, c == NC_T - 1)
                if i == NIT - 1 and c >= 8:
                    if c == 8:
                        assert not extras, f"extras must drain: {len(extras)} left"
                        po2L = [
                            xp.tile([128, 512], f32, tag="x", name=f"avL_{h2}")
                            for h2 in range(2)
                        ]
                    emit_attnv(i, c - 8, po2L, c == 8, False)
                if c == 7:
                    emit_mask(i, 0)
                elif c == NC_T - 1:
                    emit_mask(i, 1)
                # dma prefetches
                if pair == 0 and c == 2 and sb + 1 < NSB:
                    emit_qdma(sb + 1)
                elif pair == 0 and c == 4 and sb + 1 < NSB:
                    emit_mask_dma(sb + 1)
                elif i == 0 and c == 10:
                    emit_vdma(2)
                elif i == 0 and c == 12:
                    emit_vdma(3)
                # extras: ~1 unit/slot; more on the attnV-free first iteration
                if i == 0:
                    budget = 0 if c < 4 else 2
                elif i == 1:
                    budget = 2 if c < 8 else 1
                elif i == NIT - 1:
                    budget = 2 if c == 0 else (1 if c < 7 else 0)
                else:
                    budget = 1
                while budget > 0 and extras:
                    extras.pop(0)[1]()
                    budget -= 1
            if i > 0:
                emit_norm(i - 1, po2)
                if pair == 0 and sb >= 1:
                    # both pairs of sb-1 are normed now
                    for st in range(4):
                        for mt in range(2):
                            extras.append(
                                ("wo",
                                 lambda s=sb - 1, a=st, b=mt: emit_wo_group(s, a, b))
                            )
        # ---- tail: attnV(last) chunks 8-15, norm, Wo(sb3) ----
        for c in range(8, NC_T):
            emit_attnv(NIT - 1, c, po2L, False, c == NC_T - 1)
        emit_norm(NIT - 1, po2L)
        for _, fn in extras:
            fn()
        for st in range(4):
            for mt in range(2):
                emit_wo_group(NSB - 1, st, mt)


def build_nc():
    nc = bacc.Bacc("TRN2", target_bir_lowering=False, debug=False)
    names = {}

    def din(name, shape, dt):
        names[name] = nc.dram_tensor(name, shape, dt, kind="ExternalInput").ap()

    din("qT4", [NSB, 128, 8, 512], bf)
    din("kT4", [NSB, 128, 8, 512], bf)
    din("vT4", [NSB, 128, 8, 512], bf)
    din("mask4", [NSB, 128, NC_T, 512], bf)
    din("wq3", [128, 8, JC], bf)
    din("wk3", [128, 8, JC], bf)
    din("wv3", [128, 8, JC], bf)
    din("woT", [JC, D], bf)
    din("biasqk", [128, 4], f32)
    names["out_p"] = nc.dram_tensor(
        "out_p", [S, D], f32, kind="ExternalOutput"
    ).ap()
    with tile_mod.TileContext(nc) as tc:
        _emit(tc, names)
    nc.compile()
    return nc


_NC = None


def _swizzle_qkv(xT):
    # [1024, 2048] -> [4 blk, 128 p, 8 c, 512 s]; x4[b,p,c,s] = xT[c*128+p, b*512+s]
    return np.ascontiguousarray(
        xT.reshape(8, 128, 4, 512).transpose(2, 1, 0, 3)
    ).astype(BF16)


def prep_inputs(q, k, v, mask, Wq, bq, Wk, bk, Wv, bv, Wo, bo):
    q = np.asarray(q, F32)
    k = np.asarray(k, F32)
    v = np.asarray(v, F32)
    mask = np.asarray(mask)
    Wq, Wk, Wv, Wo = (np.asarray(w, F32) for w in (Wq, Wk, Wv, Wo))
    bq, bk, bv, bo = (np.asarray(b_, F32) for b_ in (bq, bk, bv, bo))

    maskT = np.ascontiguousarray(mask[0, 0].T).astype(F32)
    mask4 = np.ascontiguousarray(
        maskT.reshape(16, 128, 4, 512).transpose(2, 1, 0, 3)
    ).astype(BF16)
    q4 = [_swizzle_qkv(q[b_].T) for b_ in range(B)]
    k4 = [_swizzle_qkv(k[b_].T) for b_ in range(B)]
    v4 = [_swizzle_qkv(v[b_].T) for b_ in range(B)]

    def w3(W, js):
        # Wx[js, :].T [1024, 256] -> [128 p, 8 c, 256 j]
        return np.ascontiguousarray(
            W[js, :].T.reshape(8, 128, JC).transpose(1, 0, 2)
        ).astype(BF16)

    in_maps = []
    for cidx in range(N_CORES):
        b_, g = cidx // 4, cidx % 4
        js = slice(g * JC, (g + 1) * JC)
        biasqk = np.stack(
            [bq[js][:128], bq[js][128:], bk[js][:128], bk[js][128:]], axis=1
        ).astype(F32)
        in_maps.append(
            {
                "qT4": q4[b_],
                "kT4": k4[b_],
                "vT4": v4[b_],
                "mask4": mask4,
                "wq3": w3(Wq, js),
                "wk3": w3(Wk, js),
                "wv3": w3(Wv, js),
                "woT": np.ascontiguousarray(Wo[:, js].T).astype(BF16),
                "biasqk": np.ascontiguousarray(biasqk),
            }
        )
    # bv contributes a constant (softmax rows sum to 1): out += Wo @ bv + bo
    bias_out = (Wo @ bv + bo).astype(F32)
    return in_maps, bias_out


def run_prepped(in_maps, bias_out, trace=False, **kw):
    global _NC
    if _NC is None:
        _NC = build_nc()
    res = run_bass_kernel_spmd(
        _NC, in_maps, list(range(N_CORES)), trace=trace, **kw
    )
    out = np.zeros((B, S, D), F32)
    for c in range(N_CORES):
        out[c // 4] += res.results[c]["out_p"]
    out += bias_out[None, None, :]
    return out, res


def kernel(q, k, v, mask, Wq, bq, Wk, bk, Wv, bv, Wo, bo):
    in_maps, bias_out = prep_inputs(
        q, k, v, mask, Wq, bq, Wk, bk, Wv, bv, Wo, bo
    )
    out, _ = run_prepped(in_maps, bias_out)
    return out
